# revision 9
# baseline (speedup 1.0000x reference)
"""ADL (attention-dropout-layer) forward kernel for Trainium2, 8 NeuronCores.

Pure data parallel: batch 64 is split 8 ways; each core handles 8 samples.
Per sample: 1x1 conv (channel contraction) -> logits z [1024 spatial],
drop the top-M spatial positions (mask=0), keep bottom (1024-M) (mask=1),
output fm * mask and attn = sigmoid(z + b).

Self-contained: hardcodes shapes B,C,H,W = 64,1024,32,32 and n_cores=8.
"""

from contextlib import ExitStack

import numpy as np

import concourse.bacc as bacc
import concourse.bass as bass
import concourse.mybir as mybir
from concourse.tile import TileContext
from concourse.bass_utils import run_bass_kernel_spmd
from concourse.kernels.top_k import topk_mask

N_CORES = 8
B, C, H, W = 64, 1024, 32, 32
HW = H * W            # 1024 spatial positions
BS = B // N_CORES     # 8 samples per core
KC = C // 128         # 8 channel chunks of 128
F32 = mybir.dt.float32


def build_nc(m_drop: int, bias_val: float) -> bass.Bass:
    k_keep = HW - m_drop
    nc = bacc.Bacc(None, target_bir_lowering=False)

    fm = nc.declare_dram_parameter("feature_maps", [BS, C, H, W], F32, isOutput=False)
    w = nc.declare_dram_parameter("conv_w", [C], F32, isOutput=False)
    dropped = nc.declare_dram_parameter("dropped", [BS, C, H, W], F32, isOutput=True)
    attn = nc.declare_dram_parameter("attn", [BS, 1, H, W], F32, isOutput=True)

    fm_ap = fm[:].rearrange("b c h w -> b c (h w)")          # [8, 1024, 1024]
    drop_ap = dropped[:].rearrange("b c h w -> b c (h w)")   # [8, 1024, 1024]
    attn_ap = attn[:].rearrange("b o h w -> b (o h w)")      # [8, 1024]

    with TileContext(nc) as tc, ExitStack() as ctx:
        singles = ctx.enter_context(tc.tile_pool(name="singles", bufs=1))
        fm_in1 = ctx.enter_context(tc.tile_pool(name="fm_in1", bufs=6))
        fm_in2 = ctx.enter_context(tc.tile_pool(name="fm_in2", bufs=6))
        fm_out = ctx.enter_context(tc.tile_pool(name="fm_out", bufs=6))
        mask_sbp = ctx.enter_context(tc.tile_pool(name="mask_sb", bufs=2))
        zrow_p = ctx.enter_context(tc.tile_pool(name="zrow", bufs=2))
        psum_z = ctx.enter_context(tc.tile_pool(name="psum_z", bufs=2, space="PSUM"))
        psum_m = ctx.enter_context(tc.tile_pool(name="psum_m", bufs=2, space="PSUM"))

        # conv weight, chunked: w_sb[p, k] = w[k*128 + p]
        w_sb = singles.tile([128, KC], F32)
        nc.sync.dma_start(out=w_sb, in_=w[:].rearrange("(k p) -> p k", p=128))

        # one-hot lhsT blocks for broadcasting mask row s to 128 partitions:
        # oh128[k, s*128 + j] = 1 iff k == s. Built via affine_select:
        # iota[k, s, j] = k - s; (iota == 0) ? 1.0 : 0.0
        ones8 = singles.tile([BS, BS * 128], F32)
        nc.vector.memset(ones8, 1.0)
        oh128 = singles.tile([BS, BS * 128], F32)
        nc.gpsimd.affine_select(
            out=oh128,
            in_=ones8,
            pattern=[[-1, BS], [0, 128]],
            compare_op=mybir.AluOpType.is_equal,
            fill=0.0,
            base=0,
            channel_multiplier=1,
        )

        z_all = singles.tile([BS, HW], F32)

        # ---- stage 1: logits via PE: z[s, :] = sum_k w_chunk_k.T @ fm[s, chunk_k, :]
        for s in range(BS):
            zp = psum_z.tile([1, HW], F32)
            for k in range(KC):
                t = fm_in1.tile([128, HW], F32)
                nc.sync.dma_start(out=t, in_=fm_ap[s, k * 128 : (k + 1) * 128, :])
                for n in range(2):
                    nc.tensor.matmul(
                        zp[0:1, n * 512 : (n + 1) * 512],
                        lhsT=w_sb[:, k : k + 1],
                        rhs=t[:, n * 512 : (n + 1) * 512],
                        start=(k == 0),
                        stop=(k == KC - 1),
                    )
            # gather sample s logits onto partition s of z_all: PSUM -> SBUF
            # copy (same partition), then cross-partition DMA SBUF -> SBUF
            zr = zrow_p.tile([1, HW], F32)
            nc.scalar.copy(out=zr, in_=zp[0:1, :])
            nc.sync.dma_start(out=z_all[s : s + 1, :], in_=zr)

        # ---- stage 2: per-sample top-(1024-M) keep mask + attn output
        zmax1 = singles.tile([BS, 1], F32)
        nc.vector.tensor_reduce(
            out=zmax1, in_=z_all, axis=mybir.AxisListType.X, op=mybir.AluOpType.max
        )
        nc.vector.tensor_scalar_add(zmax1, zmax1, 1.0)
        # v = (z - (zmax+1)) * -1 = zmax + 1 - z  >= 1, order reversed vs z
        v = singles.tile([BS, HW], F32)
        nc.vector.tensor_scalar(
            out=v,
            in0=z_all,
            scalar1=zmax1,
            scalar2=-1.0,
            op0=mybir.AluOpType.subtract,
            op1=mybir.AluOpType.mult,
        )
        # keep-mask: 1 at the k_keep smallest z (= largest v), 0 at top-M of z
        mask = singles.tile([BS, HW], F32)
        topk_mask.__wrapped__(
            tc, out=mask, in_=v, k_to_choose=k_keep, ctx=ctx, min_val=0
        )

        # attn = sigmoid(z + b)
        attn_sb = singles.tile([BS, HW], F32)
        nc.scalar.activation(
            out=attn_sb,
            in_=z_all,
            func=mybir.ActivationFunctionType.Sigmoid,
            bias=bias_val,
            scale=1.0,
        )
        nc.sync.dma_start(out=attn_ap[:, :], in_=attn_sb)

        # ---- stage 3: dropped = fm * mask (mask broadcast over channels)
        for s in range(BS):
            mp = psum_m.tile([128, HW], F32)
            for n in range(2):
                nc.tensor.matmul(
                    mp[:, n * 512 : (n + 1) * 512],
                    lhsT=oh128[:, s * 128 : (s + 1) * 128],
                    rhs=mask[:, n * 512 : (n + 1) * 512],
                    start=True,
                    stop=True,
                )
            mask_sb = mask_sbp.tile([128, HW], F32)
            nc.vector.tensor_copy(out=mask_sb, in_=mp)
            for k in range(KC):
                t = fm_in2.tile([128, HW], F32)
                nc.sync.dma_start(out=t, in_=fm_ap[s, k * 128 : (k + 1) * 128, :])
                o = fm_out.tile([128, HW], F32)
                nc.vector.tensor_mul(out=o, in0=t, in1=mask_sb)
                nc.sync.dma_start(
                    out=drop_ap[s, k * 128 : (k + 1) * 128, :], in_=o
                )

    nc.compile()
    return nc


_CACHE: dict = {}


def _get_nc(m_drop: int, bias_val: float) -> bass.Bass:
    key = (m_drop, bias_val)
    if key not in _CACHE:
        _CACHE[key] = build_nc(m_drop, bias_val)
    return _CACHE[key]


def _run(feature_maps, conv_w, conv_b, M, trace=False):
    fm = np.ascontiguousarray(np.asarray(feature_maps, dtype=np.float32))
    w = np.ascontiguousarray(np.asarray(conv_w, dtype=np.float32))
    b = np.asarray(conv_b, dtype=np.float32)
    m_drop = int(M)
    nc = _get_nc(m_drop, float(b[0]))
    in_maps = [
        {"feature_maps": fm[i * BS : (i + 1) * BS], "conv_w": w}
        for i in range(N_CORES)
    ]
    res = run_bass_kernel_spmd(nc, in_maps, list(range(N_CORES)), trace=trace)
    results = res.results
    dropped = np.concatenate([results[i]["dropped"] for i in range(N_CORES)], axis=0)
    attn = np.concatenate([results[i]["attn"] for i in range(N_CORES)], axis=0)
    return (dropped, attn), res


def kernel(feature_maps, conv_w, conv_b, M):
    (dropped, attn), _ = _run(feature_maps, conv_w, conv_b, M)
    return dropped, attn
